# revision 7
# baseline (speedup 1.0000x reference)
"""Trainium2 Bass kernel for nn_Encoder (DA-RNN style input-attention LSTM encoder).

reference math per timestep t (B batch, D=81 input feats, H=128 hidden):
    hs    = concat(h, c)                       [B, 2H]
    e     = tanh(hs @ W1.T + b1) @ W2.T + b2   [B, D]
    alpha = softmax(e, axis=1)                 [B, D]
    xw    = alpha * x_t                        [B, D]
    gates = xw @ W_ih.T + b_ih + h @ W_hh.T + b_hh   (PyTorch order i,f,g,o)
    c'    = sigmoid(f)*c + sigmoid(i)*tanh(g)
    h'    = sigmoid(o)*tanh(c')
outputs: h stack [B, T, H], alpha stack [B, T, D]

Sharding: data-parallel over batch, 512 / 8 cores = 64 rows per core; weights
replicated. The T=256 recurrence is fully unrolled on-chip.

On-chip layout (per core, B=64 local rows):
  - state h, c feature-major [H=128 part, B free]; gates computed as 8 fp32
    matmuls (4x W_hh.T @ h, 4x W_ih_aug.T @ xw^T) into one PSUM tile [128, 4B]
  - sigma(x) = 0.5 + 0.5*tanh(x/2) with i,f,o weight rows pre-halved on host, so
    a single Tanh covers all gates and the ACT table never switches from
    exp_and_others (exp + tanh co-resident).
  - attention softmax batch-major [B, 81]: e = u.T @ W2.T (+ b2 via rank-1
    matmul accumulate), exp with fused accum_out sum, reciprocal_approx_fast,
    alpha written straight into the output accumulation buffer.
  - xw transposed to [81, B] via TensorE into an 82-row tile whose last row is
    ones (folds the gate bias into the W_ih_aug matmul).
  - h output accumulates feature-major [128, T*B] (host transposes at the end),
    alpha accumulates batch-major [B, T*81] (host reshape only).
"""

import sys

if "/opt/trn_rl_repo" not in sys.path:
    sys.path.insert(0, "/opt/trn_rl_repo")

import numpy as np

B, T, D, H = 512, 256, 81, 128
NCORES = 8
BL = B // NCORES          # 64 batch rows per core
TC = 64                   # timesteps per output DMA chunk
NG = 4                    # gates

_cache = {}


def build_program(t_steps=T, repeats=1):
    import concourse.bacc as bacc
    import concourse.tile as tile
    from concourse import mybir
    from contextlib import ExitStack

    f32 = mybir.dt.float32
    AF = mybir.ActivationFunctionType
    ALU = mybir.AluOpType

    nc = bacc.Bacc("TRN2", num_devices=NCORES, debug=False)

    x_d = nc.dram_tensor("x", [BL, t_steps * D], f32, kind="ExternalInput")
    w1at_d = nc.dram_tensor("w1at", [H, H], f32, kind="ExternalInput")
    w1bt_d = nc.dram_tensor("w1bt", [H, H], f32, kind="ExternalInput")
    b1_d = nc.dram_tensor("b1", [H, 1], f32, kind="ExternalInput")
    w2t_d = nc.dram_tensor("w2t", [H, D], f32, kind="ExternalInput")
    b2r_d = nc.dram_tensor("b2r", [1, D], f32, kind="ExternalInput")
    whh_d = nc.dram_tensor("whh", [H, NG * H], f32, kind="ExternalInput")
    wih_d = nc.dram_tensor("wih", [D + 1, NG * H], f32, kind="ExternalInput")
    ident_d = nc.dram_tensor("ident", [BL, BL], f32, kind="ExternalInput")
    ones1_d = nc.dram_tensor("ones1", [1, BL], f32, kind="ExternalInput")

    h_out_d = nc.dram_tensor("h_out", [H, t_steps * BL], f32, kind="ExternalOutput")
    a_out_d = nc.dram_tensor("a_out", [BL, t_steps * D], f32, kind="ExternalOutput")

    n_chunks = (t_steps + TC - 1) // TC

    with tile.TileContext(nc) as tc:
        with ExitStack() as ctx:
            singles = ctx.enter_context(tc.tile_pool(name="singles", bufs=1))
            hpool = ctx.enter_context(tc.tile_pool(name="hpool", bufs=2))
            apool = ctx.enter_context(tc.tile_pool(name="apool", bufs=2))
            temps = ctx.enter_context(tc.tile_pool(name="temps", bufs=2))
            ps_u = ctx.enter_context(tc.tile_pool(name="ps_u", bufs=1, space="PSUM"))
            ps_e = ctx.enter_context(tc.tile_pool(name="ps_e", bufs=1, space="PSUM"))
            ps_x = ctx.enter_context(tc.tile_pool(name="ps_x", bufs=1, space="PSUM"))
            # one PSUM bank per gate: interleaved accumulation groups within a
            # single bank are broken on HW (start=True clears bank state)
            ps_g = [
                ctx.enter_context(
                    tc.tile_pool(name=f"ps_g{q}", bufs=1, space="PSUM")
                )
                for q in range(NG)
            ]

            # --- load constants / inputs once ---
            x_sb = singles.tile([BL, t_steps * D], f32)
            nc.sync.dma_start(out=x_sb, in_=x_d[:, :])
            w1at = singles.tile([H, H], f32)
            nc.sync.dma_start(out=w1at, in_=w1at_d[:, :])
            w1bt = singles.tile([H, H], f32)
            nc.sync.dma_start(out=w1bt, in_=w1bt_d[:, :])
            b1 = singles.tile([H, 1], f32)
            nc.sync.dma_start(out=b1, in_=b1_d[:, :])
            w2t = singles.tile([H, D], f32)
            nc.sync.dma_start(out=w2t, in_=w2t_d[:, :])
            b2r = singles.tile([1, D], f32)
            nc.sync.dma_start(out=b2r, in_=b2r_d[:, :])
            whh = singles.tile([H, NG * H], f32)
            nc.sync.dma_start(out=whh, in_=whh_d[:, :])
            wih = singles.tile([D + 1, NG * H], f32)
            nc.sync.dma_start(out=wih, in_=wih_d[:, :])
            ident = singles.tile([BL, BL], f32)
            nc.sync.dma_start(out=ident, in_=ident_d[:, :])
            ones1 = singles.tile([1, BL], f32)
            nc.sync.dma_start(out=ones1, in_=ones1_d[:, :])

            # persistent state
            h0 = singles.tile([H, BL], f32)
            nc.vector.memset(h0, 0.0)
            c_sb = singles.tile([H, BL], f32)
            xwt = singles.tile([D + 1, BL], f32)   # row D stays ones forever
            nc.vector.memset(xwt, 1.0)

            def body(_iv=None):
                nc.vector.memset(c_sb, 0.0)
                h_prev = h0
                hbuf_prev = None
                for k in range(n_chunks):
                    tc_steps = min(TC, t_steps - k * TC)
                    hbuf = hpool.tile([H, TC * BL], f32, tag="hbuf")
                    abuf = apool.tile([BL, TC * D], f32, tag="abuf")
                    for j in range(tc_steps):
                        t = k * TC + j
                        # u = W1a @ h + W1b @ c          [H, B] psum
                        u_ps = ps_u.tile([H, BL], f32, tag="u")
                        nc.tensor.matmul(u_ps, w1at, h_prev, start=True, stop=False)
                        nc.tensor.matmul(u_ps, w1bt, c_sb, start=False, stop=True)
                        # gates (h part): 4 separate psum banks
                        g_ps = [
                            ps_g[q].tile([H, BL], f32, tag=f"g{q}", name=f"g{q}")
                            for q in range(NG)
                        ]
                        for q in range(NG):
                            nc.tensor.matmul(
                                g_ps[q],
                                whh[:, q * H : (q + 1) * H],
                                h_prev,
                                start=True,
                                stop=False,
                            )
                        # u_t = tanh(u + b1)             [H, B] sbuf
                        u_sb = temps.tile([H, BL], f32, tag="u_sb")
                        nc.scalar.activation(out=u_sb, in_=u_ps, func=AF.Tanh, bias=b1)
                        # e = u.T @ W2.T + b2 (rank-1)   [B, 81] psum
                        e_ps = ps_e.tile([BL, D], f32, tag="e")
                        nc.tensor.matmul(e_ps, u_sb, w2t, start=True, stop=False)
                        nc.tensor.matmul(e_ps, ones1, b2r, start=False, stop=True)
                        # w = exp(e), s = sum_d w        [B, 81] sbuf, [B,1]
                        w_sb = temps.tile([BL, D], f32, tag="w_sb")
                        s_sb = temps.tile([BL, 1], f32, tag="s_sb")
                        nc.scalar.activation(
                            out=w_sb, in_=e_ps, func=AF.Exp, accum_out=s_sb
                        )
                        # r = 1/s ; alpha = w * r  -> straight into output buffer
                        r_sb = temps.tile([BL, 1], f32, tag="r_sb")
                        nc.vector.reciprocal(out=r_sb, in_=s_sb)
                        a_slice = abuf[:, j * D : (j + 1) * D]
                        nc.vector.tensor_scalar_mul(out=a_slice, in0=w_sb, scalar1=r_sb)
                        # xw = alpha * x_t               [B, 81]
                        xw_sb = temps.tile([BL, D], f32, tag="xw_sb")
                        nc.vector.tensor_mul(
                            xw_sb, a_slice, x_sb[:, t * D : (t + 1) * D]
                        )
                        # xw^T via TensorE               [81, B] psum -> sbuf rows 0:81
                        x_ps = ps_x.tile([D, BL], f32, tag="x_ps")
                        nc.tensor.transpose(x_ps, xw_sb, ident)
                        nc.vector.tensor_copy(out=xwt[0:D, :], in_=x_ps)
                        # gates (x part, bias via ones row)
                        for q in range(NG):
                            nc.tensor.matmul(
                                g_ps[q],
                                wih[:, q * H : (q + 1) * H],
                                xwt,
                                start=False,
                                stop=True,
                            )
                        # t_g = tanh(gates)  (i,f,o pre-halved -> tanh(x/2))
                        t_sb = temps.tile([H, NG * BL], f32, tag="t_sb")
                        for q in range(NG):
                            nc.scalar.activation(
                                out=t_sb[:, q * BL : (q + 1) * BL],
                                in_=g_ps[q],
                                func=AF.Tanh,
                            )
                        # sig = 0.5*t + 0.5 for i,f,o
                        sig = temps.tile([H, 3 * BL], f32, tag="sig")
                        nc.vector.tensor_scalar(
                            out=sig,
                            in0=t_sb[:, 0 : 3 * BL],
                            scalar1=0.5,
                            scalar2=0.5,
                            op0=ALU.mult,
                            op1=ALU.add,
                        )
                        # c' = sig_f*c + sig_i*tanh(g)
                        fc = temps.tile([H, BL], f32, tag="fc")
                        nc.vector.tensor_mul(fc, sig[:, BL : 2 * BL], c_sb)
                        ig = temps.tile([H, BL], f32, tag="ig")
                        nc.vector.tensor_mul(ig, sig[:, 0:BL], t_sb[:, 3 * BL :])
                        nc.vector.tensor_add(c_sb, fc, ig)
                        # h' = sig_o * tanh(c')
                        tc_sb = temps.tile([H, BL], f32, tag="tc_sb")
                        nc.scalar.activation(out=tc_sb, in_=c_sb, func=AF.Tanh)
                        h_slice = hbuf[:, j * BL : (j + 1) * BL]
                        nc.vector.tensor_mul(h_slice, sig[:, 2 * BL : 3 * BL], tc_sb)
                        h_prev = h_slice
                    # chunk done: stream out
                    nc.sync.dma_start(
                        out=h_out_d[:, k * TC * BL : (k * TC + tc_steps) * BL],
                        in_=hbuf[:, 0 : tc_steps * BL],
                    )
                    nc.sync.dma_start(
                        out=a_out_d[:, k * TC * D : (k * TC + tc_steps) * D],
                        in_=abuf[:, 0 : tc_steps * D],
                    )
                    hbuf_prev = hbuf

            if repeats == 1:
                body()
            else:
                with tc.For_i(0, repeats, 1) as _i:
                    body(_i)

    nc.compile()
    return nc


def pack_weights(W1, b1, W2, b2, W_ih, W_hh, b_ih, b_hh):
    """Host-side packing. Gate col-block order [i, f, o, g]; i,f,o halved."""
    W1 = np.asarray(W1, np.float32)
    W2 = np.asarray(W2, np.float32)
    W_ih = np.asarray(W_ih, np.float32)
    W_hh = np.asarray(W_hh, np.float32)
    bias = np.asarray(b_ih, np.float32) + np.asarray(b_hh, np.float32)

    # PyTorch gate row ranges in W_ih/W_hh: i,f,g,o
    rng = {"i": slice(0, H), "f": slice(H, 2 * H), "g": slice(2 * H, 3 * H),
           "o": slice(3 * H, 4 * H)}
    order = ["i", "f", "o", "g"]
    scale = {"i": 0.5, "f": 0.5, "o": 0.5, "g": 1.0}

    whh = np.empty((H, NG * H), np.float32)
    wih = np.empty((D + 1, NG * H), np.float32)
    for q, name in enumerate(order):
        s = scale[name]
        whh[:, q * H : (q + 1) * H] = (W_hh[rng[name], :] * s).T
        wih[:D, q * H : (q + 1) * H] = (W_ih[rng[name], :] * s).T
        wih[D, q * H : (q + 1) * H] = bias[rng[name]] * s

    return {
        "w1at": np.ascontiguousarray(W1[:, :H].T),
        "w1bt": np.ascontiguousarray(W1[:, H:].T),
        "b1": np.asarray(b1, np.float32).reshape(H, 1),
        "w2t": np.ascontiguousarray(W2.T),
        "b2r": np.asarray(b2, np.float32).reshape(1, D),
        "whh": whh,
        "wih": wih,
        "ident": np.eye(BL, dtype=np.float32),
        "ones1": np.ones((1, BL), np.float32),
    }


def _get_program(t_steps, repeats=1):
    key = (t_steps, repeats)
    if key not in _cache:
        _cache[key] = build_program(t_steps, repeats)
    return _cache[key]


def run(X_input, weights, t_steps=T, repeats=1, nc=None):
    from concourse.bass_utils import run_bass_kernel_spmd

    if nc is None:
        nc = _get_program(t_steps, repeats)
    X_input = np.asarray(X_input, np.float32)
    n_b = X_input.shape[0]
    assert n_b == NCORES * BL
    in_maps = []
    for c in range(NCORES):
        xs = np.ascontiguousarray(
            X_input[c * BL : (c + 1) * BL, :t_steps, :].reshape(BL, t_steps * D)
        )
        in_maps.append({"x": xs, **weights})
    res = run_bass_kernel_spmd(nc, in_maps, core_ids=list(range(NCORES)))
    hs = []
    als = []
    for c in range(NCORES):
        h = res.results[c]["h_out"].reshape(H, t_steps, BL).transpose(2, 1, 0)
        a = res.results[c]["a_out"].reshape(BL, t_steps, D)
        hs.append(h)
        als.append(a)
    return np.concatenate(hs, 0), np.concatenate(als, 0)


def kernel(X_input, W1, b1, W2, b2, W_ih, W_hh, b_ih, b_hh):
    weights = pack_weights(W1, b1, W2, b2, W_ih, W_hh, b_ih, b_hh)
    h, a = run(np.asarray(X_input, np.float32), weights)
    return h, a


# revision 28
# speedup vs baseline: 1.1898x; 1.1898x over previous
"""Trainium2 Bass kernel for nn_Encoder (DA-RNN style input-attention LSTM encoder).

reference math per timestep t (B batch, D=81 input feats, H=128 hidden):
    hs    = concat(h, c)                       [B, 2H]
    e     = tanh(hs @ W1.T + b1) @ W2.T + b2   [B, D]
    alpha = softmax(e, axis=1)                 [B, D]
    xw    = alpha * x_t                        [B, D]
    gates = xw @ W_ih.T + b_ih + h @ W_hh.T + b_hh   (PyTorch order i,f,g,o)
    c'    = sigmoid(f)*c + sigmoid(i)*tanh(g)
    h'    = sigmoid(o)*tanh(c')
outputs: h stack [B, T, H], alpha stack [B, T, D]

Sharding: data-parallel over batch, 512 / 8 cores = 64 rows per core; weights
replicated. The T=256 recurrence is fully unrolled on-chip. The kernel is
latency-bound: wall time = T * (serial per-step chain latency), so the design
minimizes the dependency chain rather than engine throughput.

Layout/tricks:
  - state h, c feature-major [H=128 part, B free]; gates via 8 fp32 matmuls
    into one PSUM bank with strictly sequential accumulation groups
    (interleaved groups within a bank lose contributions on HW).
  - sigma(x) = 0.5 + 0.5*tanh(x/2) with i,f,o weight rows pre-halved on host so
    one Tanh covers all gates; only the exp_and_others ACT table is used. The
    0.5*t+0.5 fixup runs on ScalarE (Copy activation with scale/bias) directly
    after the tanh — no engine handoff.
  - softmax batch-major [B, 81]: the rank-1 b2 matmul opens the e PSUM group
    (dependency-free, off the chain); exp fuses the row-sum via accum_out;
    xw2 = w*x_t runs concurrently with r = 1/s; the transpose matmul uses
    diag(r) (I * r, one tensor_scalar) as rhs so normalization fuses into the
    transpose; alpha = w*r runs off the critical path on GPSIMD.
  - gate bias folded into an 82nd ones-row of the transposed xw operand.
  - h output accumulates feature-major [128, T*B] (host transposes at the end),
    alpha accumulates batch-major [B, T*81] (host reshape only).
"""

import sys

if "/opt/trn_rl_repo" not in sys.path:
    sys.path.insert(0, "/opt/trn_rl_repo")

import numpy as np

B, T, D, H = 512, 256, 81, 128
NCORES = 8
BL = B // NCORES          # 64 batch rows per core
TC = 64                   # timesteps per output DMA chunk
NG = 4                    # gates

_cache = {}


def build_program(t_steps=T, repeats=1):
    import concourse.bass as bass
    import concourse.bacc as bacc
    import concourse.tile as tile
    from concourse import mybir
    from contextlib import ExitStack

    f32 = mybir.dt.float32
    AF = mybir.ActivationFunctionType
    ALU = mybir.AluOpType

    nc = bacc.Bacc("TRN2", num_devices=NCORES, debug=False)

    x_d = nc.dram_tensor("x", [BL, t_steps * D], f32, kind="ExternalInput")
    w1at_d = nc.dram_tensor("w1at", [H, H], f32, kind="ExternalInput")
    w1bt_d = nc.dram_tensor("w1bt", [H, H], f32, kind="ExternalInput")
    b1_d = nc.dram_tensor("b1", [H, 1], f32, kind="ExternalInput")
    w2t_d = nc.dram_tensor("w2t", [H, D], f32, kind="ExternalInput")
    b2r_d = nc.dram_tensor("b2r", [1, D], f32, kind="ExternalInput")
    whh_d = nc.dram_tensor("whh", [H, NG * H], f32, kind="ExternalInput")
    wih_d = nc.dram_tensor("wih", [D + 1, NG * H], f32, kind="ExternalInput")
    ident_d = nc.dram_tensor("ident", [BL, BL], f32, kind="ExternalInput")
    ones1_d = nc.dram_tensor("ones1", [1, BL], f32, kind="ExternalInput")

    h_out_d = nc.dram_tensor("h_out", [H, t_steps * BL], f32, kind="ExternalOutput")
    a_out_d = nc.dram_tensor("a_out", [BL, t_steps * D], f32, kind="ExternalOutput")

    n_chunks = (t_steps + TC - 1) // TC

    with tile.TileContext(nc) as tc:
        with ExitStack() as ctx:
            singles = ctx.enter_context(tc.tile_pool(name="singles", bufs=1))
            hpool = ctx.enter_context(tc.tile_pool(name="hpool", bufs=2))
            apool = ctx.enter_context(tc.tile_pool(name="apool", bufs=2))
            temps = ctx.enter_context(tc.tile_pool(name="temps", bufs=2))
            ps_u = ctx.enter_context(tc.tile_pool(name="ps_u", bufs=1, space="PSUM"))
            ps_e = ctx.enter_context(tc.tile_pool(name="ps_e", bufs=1, space="PSUM"))
            ps_x = ctx.enter_context(tc.tile_pool(name="ps_x", bufs=1, space="PSUM"))
            # one PSUM bank per gate: lets each gate's tanh run concurrently
            # with the next gate's matmuls (same-bank PE-write + read is fatal,
            # and interleaved groups within one bank lose contributions)
            ps_g = [
                ctx.enter_context(
                    tc.tile_pool(name=f"ps_g{q}", bufs=1, space="PSUM")
                )
                for q in range(NG)
            ]

            # --- load constants / inputs once ---
            x_sb = singles.tile([BL, t_steps * D], f32)
            nc.sync.dma_start(out=x_sb, in_=x_d[:, :])
            w1at = singles.tile([H, H], f32)
            nc.sync.dma_start(out=w1at, in_=w1at_d[:, :])
            w1bt = singles.tile([H, H], f32)
            nc.sync.dma_start(out=w1bt, in_=w1bt_d[:, :])
            b1 = singles.tile([H, 1], f32)
            nc.sync.dma_start(out=b1, in_=b1_d[:, :])
            w2t = singles.tile([H, D], f32)
            nc.sync.dma_start(out=w2t, in_=w2t_d[:, :])
            b2r = singles.tile([1, D], f32)
            nc.sync.dma_start(out=b2r, in_=b2r_d[:, :])
            whh = singles.tile([H, NG * H], f32)
            nc.sync.dma_start(out=whh, in_=whh_d[:, :])
            wih = singles.tile([D + 1, NG * H], f32)
            nc.sync.dma_start(out=wih, in_=wih_d[:, :])
            ident = singles.tile([BL, BL], f32)
            nc.sync.dma_start(out=ident, in_=ident_d[:, :])
            ones1 = singles.tile([1, BL], f32)
            nc.sync.dma_start(out=ones1, in_=ones1_d[:, :])

            # persistent state
            h0 = singles.tile([H, BL], f32)
            nc.vector.memset(h0, 0.0)
            c_sb = singles.tile([H, BL], f32)
            xwt = singles.tile([D + 1, BL], f32)   # row D stays ones forever
            nc.vector.memset(xwt, 1.0)

            def emit_step(k, j, hbuf, abuf, h_prev):
                t = k * TC + j
                # u = W1b @ c + W1a @ h: c is ready before h at the end of the
                # previous step, so the c matmul opens the group early
                u_ps = ps_u.tile([H, BL], f32, tag="u")
                nc.tensor.matmul(u_ps, w1bt, c_sb, start=True, stop=False)
                nc.tensor.matmul(u_ps, w1at, h_prev, start=False, stop=True)
                # open the e psum group with the dependency-free rank-1 b2 term
                e_ps = ps_e.tile([BL, D], f32, tag="e")
                nc.tensor.matmul(e_ps, ones1, b2r, start=True, stop=False)
                # gate h-parts can start as soon as h is known (separate banks)
                g_ps = [
                    ps_g[q].tile([H, BL], f32, tag=f"g{q}", name=f"g{q}")
                    for q in range(NG)
                ]
                for q in range(NG):
                    nc.tensor.matmul(
                        g_ps[q], whh[:, q * H : (q + 1) * H], h_prev,
                        start=True, stop=False,
                    )
                # u_t = tanh(u + b1)
                u_sb = temps.tile([H, BL], f32, tag="u_sb")
                nc.scalar.activation(out=u_sb, in_=u_ps, func=AF.Tanh, bias=b1)
                # e += u.T @ W2.T
                nc.tensor.matmul(e_ps, u_sb, w2t, start=False, stop=True)
                # w = exp(e), s = sum_d w
                w_sb = temps.tile([BL, D], f32, tag="w_sb")
                s_sb = temps.tile([BL, 1], f32, tag="s_sb")
                nc.scalar.activation(out=w_sb, in_=e_ps, func=AF.Exp, accum_out=s_sb)
                # r = 1/s ; xw2 = w * x_t ; diag_r = I * r  (all DVE, one run)
                r_sb = temps.tile([BL, 1], f32, tag="r_sb")
                nc.vector.reciprocal(out=r_sb, in_=s_sb)
                xw2 = temps.tile([BL, D], f32, tag="xw2")
                nc.vector.tensor_mul(xw2, w_sb, x_sb[:, t * D : (t + 1) * D])
                diag_r = temps.tile([BL, BL], f32, tag="diag")
                nc.vector.tensor_scalar_mul(out=diag_r, in0=ident, scalar1=r_sb)
                # alpha = w * r: output only, off the critical path, on GPSIMD
                nc.gpsimd.tensor_scalar_mul(
                    out=abuf[:, j * D : (j + 1) * D], in0=w_sb, scalar1=r_sb
                )
                # xw^T (normalized) = xw2.T @ diag(r)   [81, B] psum
                x_ps = ps_x.tile([D, BL], f32, tag="x_ps")
                nc.tensor.matmul(x_ps, xw2, diag_r, start=True, stop=True)
                nc.vector.tensor_copy(out=xwt[0:D, :], in_=x_ps)
                # gate x-parts + per-gate tanh pipelined behind each matmul
                t_sb = temps.tile([H, NG * BL], f32, tag="t_sb")
                for q in range(NG):
                    nc.tensor.matmul(
                        g_ps[q], wih[:, q * H : (q + 1) * H], xwt,
                        start=False, stop=True,
                    )
                    nc.scalar.activation(
                        out=t_sb[:, q * BL : (q + 1) * BL],
                        in_=g_ps[q],
                        func=AF.Tanh,
                    )
                # gate block order is [i, f, g, o]: sig_if unblocks the c
                # update right after tanh_f; sig_o is only needed at the very
                # end for h' and runs in the slack
                sig = temps.tile([H, 2 * BL], f32, tag="sig")
                nc.vector.tensor_scalar(
                    out=sig,
                    in0=t_sb[:, 0 : 2 * BL],
                    scalar1=0.5,
                    scalar2=0.5,
                    op0=ALU.mult,
                    op1=ALU.add,
                )
                # c' = sig_f*c + sig_i*tanh(g), one DVE run
                fc = temps.tile([H, BL], f32, tag="fc")
                nc.vector.tensor_mul(fc, sig[:, BL : 2 * BL], c_sb)
                ig = temps.tile([H, BL], f32, tag="ig")
                nc.vector.tensor_mul(ig, sig[:, 0:BL], t_sb[:, 2 * BL : 3 * BL])
                sig_o = temps.tile([H, BL], f32, tag="sig_o")
                nc.vector.tensor_scalar(
                    out=sig_o,
                    in0=t_sb[:, 3 * BL :],
                    scalar1=0.5,
                    scalar2=0.5,
                    op0=ALU.mult,
                    op1=ALU.add,
                )
                nc.vector.tensor_add(c_sb, fc, ig)
                # h' = sig_o * tanh(c')
                tc_sb = temps.tile([H, BL], f32, tag="tc_sb")
                nc.scalar.activation(out=tc_sb, in_=c_sb, func=AF.Tanh)
                h_slice = hbuf[:, j * BL : (j + 1) * BL]
                nc.vector.tensor_mul(h_slice, sig_o, tc_sb)
                return h_slice

            def body(_iv=None):
                nc.vector.memset(c_sb, 0.0)
                h_prev = h0
                for k in range(n_chunks):
                    tc_steps = min(TC, t_steps - k * TC)
                    hbuf = hpool.tile([H, TC * BL], f32, tag="hbuf")
                    abuf = apool.tile([BL, TC * D], f32, tag="abuf")
                    for j in range(tc_steps):
                        h_prev = emit_step(k, j, hbuf, abuf, h_prev)
                    nc.sync.dma_start(
                        out=h_out_d[:, k * TC * BL : (k * TC + tc_steps) * BL],
                        in_=hbuf[:, 0 : tc_steps * BL],
                    )
                    nc.sync.dma_start(
                        out=a_out_d[:, k * TC * D : (k * TC + tc_steps) * D],
                        in_=abuf[:, 0 : tc_steps * D],
                    )

            if repeats == 1:
                body()
            else:
                with tc.For_i(0, repeats, 1) as _i:
                    body(_i)

    nc.compile()
    return nc


def pack_weights(W1, b1, W2, b2, W_ih, W_hh, b_ih, b_hh):
    """Host-side packing. Gate col-block order [i, f, o, g]; i,f,o halved."""
    W1 = np.asarray(W1, np.float32)
    W2 = np.asarray(W2, np.float32)
    W_ih = np.asarray(W_ih, np.float32)
    W_hh = np.asarray(W_hh, np.float32)
    bias = np.asarray(b_ih, np.float32) + np.asarray(b_hh, np.float32)

    # PyTorch gate row ranges in W_ih/W_hh: i,f,g,o
    rng = {"i": slice(0, H), "f": slice(H, 2 * H), "g": slice(2 * H, 3 * H),
           "o": slice(3 * H, 4 * H)}
    order = ["i", "f", "g", "o"]
    scale = {"i": 0.5, "f": 0.5, "o": 0.5, "g": 1.0}

    whh = np.empty((H, NG * H), np.float32)
    wih = np.empty((D + 1, NG * H), np.float32)
    for q, name in enumerate(order):
        s = scale[name]
        whh[:, q * H : (q + 1) * H] = (W_hh[rng[name], :] * s).T
        wih[:D, q * H : (q + 1) * H] = (W_ih[rng[name], :] * s).T
        wih[D, q * H : (q + 1) * H] = bias[rng[name]] * s

    return {
        "w1at": np.ascontiguousarray(W1[:, :H].T),
        "w1bt": np.ascontiguousarray(W1[:, H:].T),
        "b1": np.asarray(b1, np.float32).reshape(H, 1),
        "w2t": np.ascontiguousarray(W2.T),
        "b2r": np.asarray(b2, np.float32).reshape(1, D),
        "whh": whh,
        "wih": wih,
        "ident": np.eye(BL, dtype=np.float32),
        "ones1": np.ones((1, BL), np.float32),
    }


def _get_program(t_steps, repeats=1):
    key = (t_steps, repeats)
    if key not in _cache:
        _cache[key] = build_program(t_steps, repeats)
    return _cache[key]


def run(X_input, weights, t_steps=T, repeats=1, nc=None):
    from concourse.bass_utils import run_bass_kernel_spmd

    if nc is None:
        nc = _get_program(t_steps, repeats)
    X_input = np.asarray(X_input, np.float32)
    n_b = X_input.shape[0]
    assert n_b == NCORES * BL
    in_maps = []
    for c in range(NCORES):
        xs = np.ascontiguousarray(
            X_input[c * BL : (c + 1) * BL, :t_steps, :].reshape(BL, t_steps * D)
        )
        in_maps.append({"x": xs, **weights})
    res = run_bass_kernel_spmd(nc, in_maps, core_ids=list(range(NCORES)))
    hs = []
    als = []
    for c in range(NCORES):
        h = res.results[c]["h_out"].reshape(H, t_steps, BL).transpose(2, 1, 0)
        a = res.results[c]["a_out"].reshape(BL, t_steps, D)
        hs.append(h)
        als.append(a)
    return np.concatenate(hs, 0), np.concatenate(als, 0)


def kernel(X_input, W1, b1, W2, b2, W_ih, W_hh, b_ih, b_hh):
    weights = pack_weights(W1, b1, W2, b2, W_ih, W_hh, b_ih, b_hh)
    h, a = run(np.asarray(X_input, np.float32), weights)
    return h, a
